# revision 4
# baseline (speedup 1.0000x reference)
"""VQ codebook (KMeans EMA) TRN2 Bass kernel — data-parallel over N on 8 cores.

Self-contained: hardcodes shapes N=131072, D=512, K=512, 8 cores.

Math: argmin_k ||xn - c_k||^2 with xn = x/|x|  ==  argmax_k V[n,k],
  V[n,k] = x.c_k - 0.5*|x|*|c_k|^2   (monotonic rescale by |x|/2 > 0)
x is fed as lossless fp16 hi+lo split (host re-encoding); scores use
3 fp16 matmul products (xh ch + xl ch + xh cl), dw uses 2 (oh xh + oh xl).
quantized rows come from an indirect-DMA gather of centroid rows.
dw/counts are AllReduced on-device; EMA tail computed replicated.
"""
import sys
sys.path.insert(0, "/opt/trn_rl_repo")
import os
import numpy as np
from contextlib import ExitStack

import concourse.bass as bass
import concourse.bacc as bacc
import concourse.tile as tile
import concourse.mybir as mybir
from concourse.bass_utils import run_bass_kernel_spmd

N = 131072
D = 512
K = 512
NCORES = 8
DECAY = 0.99
EPSILON = 1e-05

f32 = mybir.dt.float32
f16 = mybir.dt.float16
u32 = mybir.dt.uint32
Alu = mybir.AluOpType

DW_X2 = os.environ.get("VQ_DW_MODE", "f16x2") == "f16x2"

_CACHED = {}


def build(ncores=NCORES, nsh=None):
    nsh = nsh or (N // ncores)
    nt = nsh // 128
    nc = bacc.Bacc("TRN2", target_bir_lowering=False, debug=False)

    # ---- per-core inputs ----
    xh_d = nc.declare_dram_parameter("xh", [nsh, D], f16, isOutput=False)
    xl_d = nc.declare_dram_parameter("xl", [nsh, D], f16, isOutput=False)
    nhn_d = nc.declare_dram_parameter("nhn", [nsh, 1], f32, isOutput=False)
    cen_d = nc.declare_dram_parameter("cen", [K, D], f32, isOutput=False)
    cth_d = nc.declare_dram_parameter("cth", [D, K], f16, isOutput=False)
    ctl_d = nc.declare_dram_parameter("ctl", [D, K], f16, isOutput=False)
    ccb_d = nc.declare_dram_parameter("ccb", [128, K], f32, isOutput=False)
    iob_d = nc.declare_dram_parameter("iob", [128, K], f32, isOutput=False)
    idm_d = nc.declare_dram_parameter("idm", [128, 128], f16, isOutput=False)
    emaw_d = nc.declare_dram_parameter("emaw", [K, D], f32, isOutput=False)
    emas_d = nc.declare_dram_parameter("emas", [1, K], f32, isOutput=False)
    ones_d = nc.declare_dram_parameter("onescol", [128, 1], f16, isOutput=False)

    # ---- outputs ----
    q_d = nc.declare_dram_parameter("q", [nsh, D], f32, isOutput=True)
    idx_d = nc.declare_dram_parameter("idx", [nsh, 1], u32, isOutput=True)
    ncen_d = nc.declare_dram_parameter("ncen", [K, D], f32, isOutput=True)
    nsize_d = nc.declare_dram_parameter("nsize", [1, K], f32, isOutput=True)
    nemaw_d = nc.declare_dram_parameter("nemaw", [K, D], f32, isOutput=True)

    # internal DRAM: dw 4x[128,D] + counts row packed -> [513, D]
    red_in = nc.dram_tensor("red_in", [513, D], f32)
    red_out = nc.dram_tensor("red_out", [513, D], f32, addr_space="Shared")
    nszs = nc.dram_tensor("nszs", [K], f32)

    xh_t = xh_d[:].rearrange("(t p) d -> t p d", p=128)
    xl_t = xl_d[:].rearrange("(t p) d -> t p d", p=128)
    nhn_t = nhn_d[:].rearrange("(t p) o -> t p o", p=128)
    q_t = q_d[:].rearrange("(t p) d -> t p d", p=128)
    idx_t = idx_d[:].rearrange("(t p) o -> t p o", p=128)

    with tile.TileContext(nc) as tc, ExitStack() as ctx:
        const = ctx.enter_context(tc.tile_pool(name="const", bufs=1))
        xin = ctx.enter_context(tc.tile_pool(name="xin", bufs=3))
        work = ctx.enter_context(tc.tile_pool(name="work", bufs=3))
        outp = ctx.enter_context(tc.tile_pool(name="outp", bufs=3))
        acc = ctx.enter_context(tc.tile_pool(name="acc", bufs=1))
        pst = ctx.enter_context(tc.tile_pool(name="pst", bufs=1, space="PSUM"))
        psv = ctx.enter_context(tc.tile_pool(name="psv", bufs=2, space="PSUM"))
        psd = ctx.enter_context(tc.tile_pool(name="psd", bufs=1, space="PSUM"))
        pse = ctx.enter_context(tc.tile_pool(name="pse", bufs=1, space="PSUM"))

        # ---- static tiles ----
        t_cth = const.tile([128, 4, K], f16, tag="cth")
        nc.sync.dma_start(t_cth[:], cth_d[:].rearrange("(c p) k -> p c k", p=128))
        t_ctl = const.tile([128, 4, K], f16, tag="ctl")
        nc.sync.dma_start(t_ctl[:], ctl_d[:].rearrange("(c p) k -> p c k", p=128))
        t_ccb = const.tile([128, K], f32, tag="ccb")
        nc.sync.dma_start(t_ccb[:], ccb_d[:])
        t_iob = const.tile([128, K], f32, tag="iob")
        nc.sync.dma_start(t_iob[:], iob_d[:])
        t_idm = const.tile([128, 128], f16, tag="idm")
        nc.sync.dma_start(t_idm[:], idm_d[:])
        t_ones = const.tile([128, 1], f16, tag="ones")
        nc.sync.dma_start(t_ones[:], ones_d[:])

        # persistent accumulators
        dw_ps = psd.tile([128, 4, 512], f32, tag="dwacc")  # 4 banks (k-chunks)
        cnt_acc = acc.tile([128, K], f16, tag="cntacc")
        nc.gpsimd.memset(cnt_acc[:], 0.0)

        for i in range(nt):
            # ---- load ----
            t_xh = xin.tile([128, D], f16, tag="xh")
            nc.sync.dma_start(t_xh[:], xh_t[i, :, :])
            t_xl = xin.tile([128, D], f16, tag="xl")
            nc.sync.dma_start(t_xl[:], xl_t[i, :, :])
            t_nhn = xin.tile([128, 1], f32, tag="nhn")
            nc.sync.dma_start(t_nhn[:], nhn_t[i, :, :])

            # ---- transpose xh, xl (PE) into one psum bank, evac via ACT ----
            tp = pst.tile([128, 2, 4, 128], f16, tag="tp")
            for s, src in enumerate((t_xh, t_xl)):
                for c in range(4):
                    nc.tensor.matmul(
                        tp[:, s, c, :], lhsT=src[:, bass.ts(c, 128)], rhs=t_idm[:],
                        is_transpose=True,
                        start=(s == 0 and c == 0), stop=(s == 1 and c == 3))
            t_xt = work.tile([128, 2, 4, 128], f16, tag="xt")
            nc.scalar.copy(t_xt[:], tp[:])

            # ---- score matmuls: vp[n,k] += x[n,d] ct[d,k] (3 fp16 products) ----
            vp = psv.tile([128, K], f32, tag="vps")
            pairs = [(0, t_cth), (1, t_cth), (0, t_ctl)]
            nmm = len(pairs) * 4
            j = 0
            for (xi, ct) in pairs:
                for c in range(4):
                    nc.tensor.matmul(
                        vp[:], lhsT=t_xt[:, xi, c, :], rhs=ct[:, c, :],
                        start=(j == 0), stop=(j == nmm - 1))
                    j += 1

            # ---- V = (ccb * nhn) + T  (nhn = -0.5|x|), argmax via max8 ----
            t_v = work.tile([128, K], f32, tag="v")
            nc.vector.scalar_tensor_tensor(
                out=t_v[:], in0=t_ccb[:], scalar=t_nhn[:], in1=vp[:],
                op0=Alu.mult, op1=Alu.add)
            t_max8 = work.tile([128, 8], f32, tag="max8")
            nc.vector.max(t_max8[:], t_v[:])
            t_idx8 = work.tile([128, 8], u32, tag="idx8")
            nc.vector.max_index(t_idx8[:], t_max8[:], t_v[:])
            nc.sync.dma_start(idx_t[i, :, :], t_idx8[:, 0:1])

            # ---- onehot (f16) from idx ----
            t_idxf = work.tile([128, 1], f32, tag="idxf")
            nc.vector.tensor_copy(t_idxf[:], t_idx8[:, 0:1])
            t_oh = work.tile([128, K], f16, tag="oh")
            nc.vector.tensor_scalar(t_oh[:], t_iob[:], t_idxf[:], None, Alu.is_equal)

            # ---- counts accumulate on gpsimd (f16 exact; max count 128<2048) ----
            nc.gpsimd.tensor_tensor(cnt_acc[:], cnt_acc[:], t_oh[:], Alu.add)

            # ---- dw[kc,d] += oh[n,kc]^T x[n,d] ----
            dwsrc = (t_xh, t_xl) if DW_X2 else (t_xh,)
            for c in range(4):
                for si, xsrc in enumerate(dwsrc):
                    nc.tensor.matmul(
                        dw_ps[:, c, :], lhsT=t_oh[:, bass.ts(c, 128)], rhs=xsrc[:],
                        start=(i == 0 and si == 0),
                        stop=(i == nt - 1 and si == len(dwsrc) - 1))

            # ---- quantized gather + store ----
            t_q = outp.tile([128, D], f32, tag="q")
            nc.gpsimd.indirect_dma_start(
                out=t_q[:], out_offset=None, in_=cen_d[:],
                in_offset=bass.IndirectOffsetOnAxis(ap=t_idx8[:, 0:1], axis=0))
            nc.sync.dma_start(q_t[i, :, :], t_q[:])

        # ================= tail =================
        cntrow_ps = pse.tile([1, K], f32, tag="cntrow")
        nc.tensor.matmul(cntrow_ps[:], lhsT=t_ones[:], rhs=cnt_acc[:],
                         start=True, stop=True)

        t_dw = acc.tile([128, 4, D], f32, tag="dwsb")
        nc.vector.tensor_copy(t_dw[:], dw_ps[:])
        t_cnt = acc.tile([1, K], f32, tag="cntsb")
        nc.vector.tensor_copy(t_cnt[:], cntrow_ps[:])

        t_dws = acc.tile([128, 4, D], f32, tag="dwsum")
        t_cnts = acc.tile([1, K], f32, tag="cntsum")
        with tc.tile_critical():
            dsem = nc.alloc_semaphore("dsem")
            csem = nc.alloc_semaphore("csem")
            nc.gpsimd.dma_start(
                red_in[0:512, :].rearrange("(c p) d -> p c d", p=128), t_dw[:]
            ).then_inc(dsem, 16)
            nc.gpsimd.dma_start(red_in[512:513, :], t_cnt[:]).then_inc(dsem, 16)
            nc.gpsimd.wait_ge(dsem, 32)
            nc.gpsimd.collective_compute(
                "AllReduce", Alu.add,
                replica_groups=[list(range(ncores))],
                ins=[red_in[:]], outs=[red_out[:]],
            ).then_inc(csem, 1)
            nc.gpsimd.wait_ge(csem, 1)
            nc.gpsimd.dma_start(
                t_dws[:], red_out[0:512, :].rearrange("(c p) d -> p c d", p=128)
            ).then_inc(dsem, 16)
            nc.gpsimd.dma_start(t_cnts[:], red_out[512:513, :]).then_inc(dsem, 16)
            nc.gpsimd.wait_ge(dsem, 64)

        # ---- new_size (row layout [1, K]) ----
        t_emas = acc.tile([1, K], f32, tag="emas")
        nc.sync.dma_start(t_emas[:], emas_d[:])
        tmp = acc.tile([1, K], f32, tag="nstmp")
        nc.vector.tensor_scalar(tmp[:], t_cnts[:], float(1.0 - DECAY), None, Alu.mult)
        ns0 = acc.tile([1, K], f32, tag="ns0")
        nc.vector.scalar_tensor_tensor(
            out=ns0[:], in0=t_emas[:], scalar=float(DECAY), in1=tmp[:],
            op0=Alu.mult, op1=Alu.add)
        t_n = acc.tile([1, 1], f32, tag="nsum")
        nc.vector.tensor_reduce(t_n[:], ns0[:], axis=mybir.AxisListType.X, op=Alu.add)
        t_nk = acc.tile([1, 1], f32, tag="nk")
        nc.vector.tensor_scalar(t_nk[:], t_n[:], float(K * EPSILON), None, Alu.add)
        t_rnk = acc.tile([1, 1], f32, tag="rnk")
        nc.vector.reciprocal(t_rnk[:], t_nk[:])
        t_alpha = acc.tile([1, 1], f32, tag="alpha")
        nc.vector.tensor_tensor(t_alpha[:], t_n[:], t_rnk[:], Alu.mult)
        t_nsize = acc.tile([1, K], f32, tag="nsize")
        nc.vector.tensor_scalar(t_nsize[:], ns0[:], float(EPSILON), t_alpha[:],
                                Alu.add, Alu.mult)
        nc.sync.dma_start(nsize_d[:], t_nsize[:])

        # roundtrip nsize through DRAM to get column layout [128, 4]
        # (explicit sems: Tile does not track DRAM-aliasing deps)
        t_nszT = acc.tile([128, 4], f32, tag="nszT")
        with tc.tile_critical():
            rsem = nc.alloc_semaphore("rsem")
            nc.gpsimd.dma_start(nszs[:], t_nsize[0:1, :]).then_inc(rsem, 16)
            nc.gpsimd.wait_ge(rsem, 16)
            nc.gpsimd.dma_start(
                t_nszT[:], nszs[:].rearrange("(c p) -> p c", p=128)
            ).then_inc(rsem, 16)
            nc.gpsimd.wait_ge(rsem, 32)
        t_rsz = acc.tile([128, 4], f32, tag="rsz")
        nc.vector.reciprocal(t_rsz[:], t_nszT[:])

        # ---- new_ema_w = emaw*DECAY + dw*(1-DECAY); new_cen = new_ema_w * rsz ----
        emaw_r = emaw_d[:].rearrange("(c p) d -> c p d", p=128)
        nemaw_r = nemaw_d[:].rearrange("(c p) d -> c p d", p=128)
        ncen_r = ncen_d[:].rearrange("(c p) d -> c p d", p=128)
        for c in range(4):
            t_ew = acc.tile([128, D], f32, tag="ew")
            nc.sync.dma_start(t_ew[:], emaw_r[c, :, :])
            t_dwc = acc.tile([128, D], f32, tag="dwc")
            nc.vector.tensor_scalar(t_dwc[:], t_dws[:, c, :], float(1.0 - DECAY),
                                    None, Alu.mult)
            t_new = acc.tile([128, D], f32, tag="newew")
            nc.vector.scalar_tensor_tensor(
                out=t_new[:], in0=t_ew[:], scalar=float(DECAY), in1=t_dwc[:],
                op0=Alu.mult, op1=Alu.add)
            nc.sync.dma_start(nemaw_r[c, :, :], t_new[:])
            t_ncen = acc.tile([128, D], f32, tag="ncen")
            nc.vector.tensor_scalar(t_ncen[:], t_new[:], t_rsz[:, c:c + 1],
                                    None, Alu.mult)
            nc.sync.dma_start(ncen_r[c, :, :], t_ncen[:])

    nc.compile()
    return nc


def _prep(X, centroids, ema_cluster_size, ema_w, ncores=NCORES):
    X = np.ascontiguousarray(X, dtype=np.float32)
    C = np.ascontiguousarray(centroids, dtype=np.float32)
    xh = X.astype(np.float16)
    xl = (X - xh.astype(np.float32)).astype(np.float16)
    sumsq = np.einsum("nd,nd->n", X.astype(np.float64), X.astype(np.float64))
    nhn = (-0.5 * np.sqrt(sumsq)).astype(np.float32).reshape(-1, 1)
    CT = np.ascontiguousarray(C.T)
    cth = CT.astype(np.float16)
    ctl = (CT - cth.astype(np.float32)).astype(np.float16)
    cc = np.einsum("kd,kd->k", C.astype(np.float64),
                   C.astype(np.float64)).astype(np.float32)
    ccb = np.ascontiguousarray(np.broadcast_to(cc[None, :], (128, K)))
    iob = np.ascontiguousarray(
        np.broadcast_to(np.arange(K, dtype=np.float32)[None, :], (128, K)))
    idm = np.eye(128, dtype=np.float16)
    ones = np.ones((128, 1), np.float16)
    emas = np.ascontiguousarray(ema_cluster_size, dtype=np.float32).reshape(1, K)
    emaw = np.ascontiguousarray(ema_w, dtype=np.float32)

    nsh = X.shape[0] // ncores
    in_maps = []
    for c in range(ncores):
        sl = slice(c * nsh, (c + 1) * nsh)
        in_maps.append(dict(
            xh=xh[sl], xl=xl[sl], nhn=nhn[sl], cen=C, cth=cth, ctl=ctl,
            ccb=ccb, iob=iob, idm=idm, emaw=emaw, emas=emas, onescol=ones))
    return in_maps


def kernel(X, centroids, ema_cluster_size, ema_w, _trace=False, _trace_kwargs=None):
    in_maps = _prep(np.asarray(X), np.asarray(centroids),
                    np.asarray(ema_cluster_size), np.asarray(ema_w))
    if "nc" not in _CACHED:
        _CACHED["nc"] = build()
    nc = _CACHED["nc"]
    kw = dict(trace=True, **(_trace_kwargs or {})) if _trace else {}
    rr = run_bass_kernel_spmd(nc, in_maps, list(range(NCORES)), **kw)
    res = rr.results
    quant = np.concatenate([res[c]["q"] for c in range(NCORES)], axis=0)
    idx = np.concatenate([res[c]["idx"] for c in range(NCORES)],
                         axis=0).astype(np.int32)
    ncen = res[0]["ncen"]
    nsize = res[0]["nsize"].reshape(K)
    nemaw = res[0]["nemaw"]
    kernel._last = rr
    return quant, idx, ncen, nsize, nemaw
